# revision 1
# baseline (speedup 1.0000x reference)
"""Trainium2 Bass kernel for nn_AttentionModuleEx1 (LKA-style attention module).

Per-sample computation (512 ch, 64x64 spatial):
  attn = dw5x5(x) + b0
  a_i  = dwH(dwW(attn)) dilated separable branches (k=7,11,21, dil=3)
  s    = attn + a0 + a1 + a2
  y    = (W3 @ s + b3') * x           (1x1 pointwise conv over channels;
                                       b3' folds the branch-bias vector)

Sharding: pure data-parallel - batch 8 -> 1 sample per NeuronCore.

Implementation:
  - channels on partitions (4 blocks of 128), spatial on the free dim.
  - 5x5 depthwise: fp16 diagonal matmuls on PE (SBUF-resident diag stacks,
    one batched DMA per block) + taps offloaded to the DVE (tensor_scalar +
    add) and ACT (activation-scale, DVE add) lanes, chained into the
    accumulator at 2048-element chunks to keep dependency chains short.
    The per-block PE/DVE/ACT split is tuned so later blocks' DVE chains
    hide behind earlier blocks' branch PE work (K5B).
  - dilated separable branches: fp8e4 DoubleRow diagonal matmuls (two taps
    per PE instruction at 0.5 cycles/row). Branch weights are scaled x16
    into fp8's normal range; branch canvases (attn, z7/z11/z21) are fp8.
    The H-conv PSUM (x256) is folded into the accumulator with one DVE
    scalar_tensor_tensor per chunk (scale 1/256, no intermediate buffer).
  - folded branch-bias: b3' = b3 + W3 @ bsumh on the host, so no bias work
    on device beyond the two Act evictions (b0, b3').
  - pointwise conv: fp16 PE matmuls (lhsT = W3^T row-tiles); bias via ACT
    eviction, multiply-by-x on DVE, one batched output DMA per block.
  - block emission is interleaved (A0 A1 B0 A2 B1 A3 B2 B3) so stage-B PE
    work overlaps the serial lane-accumulation chains of other blocks.
"""

import os
import sys

for p in ("/opt/trn_rl_repo", "/opt/pypackages"):
    if p not in sys.path:
        sys.path.insert(0, p)

import numpy as np

C, H, W = 512, 64, 64
NBLK = 4
P = 128
HW = H * W

# 5x5 tap split per block (PE, DVE, ACT per 128-ch block; each sums to 25).
# DVE-lane chains accumulate independently of the PSUM eviction, so a
# uniform split schedules best (tuned via TimelineSim sweep).
K5B = [[int(v) for v in grp.split(",")]
       for grp in os.environ.get(
           "K5B", "13,12,0;13,12,0;13,12,0;13,12,0").split(";")]
assert len(K5B) == NBLK and all(sum(g) == 25 for g in K5B)
BR_SCALE = 16.0  # branch weights are stored x16 in fp8

# (branch, taps, pairs, W-pad, z rows incl. OOB guard, z lead pad)
WBR = (("7", 7, 4, 9, 85, 9), ("11", 11, 6, 15, 97, 15),
       ("21", 21, 11, 30, 127, 30))

_NC = None


def _build_nc():
    import concourse.bass as bass  # noqa: F401
    import concourse.bacc as bacc
    import concourse.mybir as mybir
    from concourse.bass import AP
    from concourse.tile import TileContext

    f32 = mybir.dt.float32
    f16 = mybir.dt.float16
    f8 = mybir.dt.float8e4
    A = mybir.AluOpType
    AF = mybir.ActivationFunctionType
    DR = mybir.MatmulPerfMode.DoubleRow

    nc = bacc.Bacc(None, target_bir_lowering=False)

    xc_d = nc.dram_tensor("xcanv", [C, 68, 68], f16, kind="ExternalInput")
    b0_d = nc.dram_tensor("b0", [C, 1], f32, kind="ExternalInput")
    w0sc_d = nc.dram_tensor("w0sc", [C, 25], f32, kind="ExternalInput")
    wd5_d = nc.dram_tensor("wd5", [NBLK, P, 25, P], f16, kind="ExternalInput")
    brd = {}
    for nm, k, npair, wpad, zrows, zpad in WBR:
        brd["w" + nm] = nc.dram_tensor(
            "wdw" + nm, [NBLK, P, npair, 2, P], f8, kind="ExternalInput")
        brd["h" + nm] = nc.dram_tensor(
            "wdh" + nm, [NBLK, P, npair, 2, P], f8, kind="ExternalInput")
    w3_d = nc.dram_tensor("w3", [C, C], f16, kind="ExternalInput")  # W3^T
    b3_d = nc.dram_tensor("b3", [C, 1], f32, kind="ExternalInput")
    out_d = nc.dram_tensor("out", [C, H, W], f16, kind="ExternalOutput")

    with TileContext(nc) as tc:
        with tc.tile_pool(name="main", bufs=1) as MP, \
             tc.tile_pool(name="canv", bufs=2) as CP, \
             tc.tile_pool(name="attn", bufs=4) as APool, \
             tc.tile_pool(name="psum", bufs=2, space="PSUM") as PP, \
             tc.tile_pool(name="diag", bufs=2) as DP, \
             tc.tile_pool(name="bdiag", bufs=int(os.environ.get("BDPB", "2"))) as BDP, \
             tc.tile_pool(name="stage", bufs=2) as SP:

            accs = [MP.tile([P, HW], f16, tag=f"acc{b}", name=f"acc{b}")
                    for b in range(NBLK)]
            attn8s = [APool.tile([P, 64, 128], f8, tag="attn8",
                                 name=f"attn8_{b}") for b in range(NBLK)]

            # SBUF-resident pointwise weights: 4 row-bands of W3^T
            w3rows = []
            for kk in range(NBLK):
                t = MP.tile([P, C], f16, tag=f"w3r{kk}", name=f"w3r{kk}")
                nc.sync.dma_start(t[:, :], w3_d[kk * P:(kk + 1) * P, :])
                w3rows.append(t)

            wtiles = {}
            for b in range(NBLK):
                sl = slice(b * P, (b + 1) * P)
                for nm, dd, k in (("b0", b0_d, 1), ("b3", b3_d, 1),
                                  ("w0sc", w0sc_d, 25)):
                    t = MP.tile([P, k], f32, tag=f"{nm}_{b}", name=f"{nm}_{b}")
                    nc.sync.dma_start(t[:, :], dd[sl, :])
                    wtiles[(nm, b)] = t

            # ---------------- stage A: 5x5 depthwise -----------------------
            def stage_a(b):
                sl = slice(b * P, (b + 1) * P)
                xcan = CP.tile([P, 68, 68], f16, tag="xcan", name="xcan")
                nc.sync.dma_start(xcan[:, :, :], xc_d[sl, :, :])
                n_pe, n_dve, n_act = K5B[b]
                d5 = DP.tile([P, n_pe, P], f16, tag="d5", name="d5")
                nc.sync.dma_start(d5[:, :, :], wd5_d[b][:, 0:n_pe, :])

                acc3 = accs[b].rearrange("p (a b) -> p a b", a=H)
                attn8 = attn8s[b]
                nc.gpsimd.memset(attn8[:, :, 0:30], 0.0)
                nc.gpsimd.memset(attn8[:, :, 94:128], 0.0)

                def rv5(t, r0, r1):
                    dh, dw = t // 5, t % 5
                    return xcan[:, dh + r0:dh + r1, dw:dw + 64]

                w0sc = wtiles[("w0sc", b)]
                for c in range(2):
                    ps = PP.tile([P, 2048], f32, tag="ps", name="ps")
                    for i in range(n_pe):
                        for j in range(4):
                            r0 = 32 * c + 8 * j
                            nc.tensor.matmul(
                                ps[:, 512 * j:512 * (j + 1)], d5[:, i, :],
                                rv5(i, r0, r0 + 8),
                                start=(i == 0), stop=(i == n_pe - 1))
                    ps3 = ps.rearrange("p (a b) -> p a b", a=32)
                    nc.scalar.activation(
                        acc3[:, 32 * c:32 * c + 32, :], ps3[:, :, :],
                        AF.Identity, bias=wtiles[("b0", b)][:, 0:1],
                        scale=1.0)

                    acc_c = accs[b][:, 2048 * c:2048 * (c + 1)]
                    if n_dve:
                        # DVE lane accumulates into its own tile so the chain
                        # depends only on xcan, not on the PSUM eviction
                        lacc = SP.tile([P, 2048], f16, tag="lacc", bufs=2,
                                       name="lacc")
                        for i in range(n_dve):
                            t = n_pe + i
                            if i == 0:
                                nc.vector.tensor_scalar_mul(
                                    lacc.rearrange("p (a b) -> p a b", a=32),
                                    rv5(t, 32 * c, 32 * c + 32),
                                    w0sc[:, t:t + 1])
                                continue
                            tmp = SP.tile([P, 2048], f16, tag="ttmp", bufs=3,
                                          name="ttmp")
                            nc.vector.tensor_scalar_mul(
                                tmp.rearrange("p (a b) -> p a b", a=32),
                                rv5(t, 32 * c, 32 * c + 32), w0sc[:, t:t + 1])
                            nc.vector.tensor_tensor(lacc[:, :], tmp[:, :],
                                                    lacc[:, :], op=A.add)
                        nc.vector.tensor_tensor(acc_c, lacc[:, :], acc_c,
                                                op=A.add)
                    for i in range(n_act):
                        # ACT lane: multiply on ACT, accumulate on DVE
                        t = n_pe + n_dve + i
                        tmp = SP.tile([P, 2048], f16, tag="ttmp", bufs=3,
                                      name="ttmp")
                        nc.scalar.activation(
                            tmp.rearrange("p (a b) -> p a b", a=32),
                            rv5(t, 32 * c, 32 * c + 32), AF.Identity,
                            bias=0.0, scale=w0sc[:, t:t + 1])
                        nc.vector.tensor_tensor(acc_c, tmp[:, :], acc_c,
                                                op=A.add)
                    # fp8 mirror of this chunk for the branch convs
                    nc.vector.tensor_scalar_mul(
                        attn8[:, 32 * c:32 * c + 32, 30:94],
                        acc3[:, 32 * c:32 * c + 32, :], 1.0)

            # --------------- stage B: dilated branches ---------------------
            def stage_b(b, first):
                attn8 = attn8s[b]
                zts = []
                for nm, k, npair, wpad, zrows, zpad in WBR:
                    zt = CP.tile([P, zrows, 64], f8, tag=f"z{nm}",
                                 name=f"z{nm}")
                    if first:
                        nc.gpsimd.memset(zt[:, 0:zpad, :], 0.0)
                        nc.gpsimd.memset(zt[:, zpad + 64:zrows, :], 0.0)
                    zts.append(zt)
                dsw, dsh = [], []
                for nm, k, npair, wpad, zrows, zpad in WBR:
                    dw_t = BDP.tile([P, npair, 2, P], f8, tag=f"dw{nm}",
                                    name=f"dw{nm}")
                    nc.sync.dma_start(dw_t[:, :, :, :], brd["w" + nm][b])
                    dh_t = BDP.tile([P, npair, 2, P], f8, tag=f"dh{nm}",
                                    name=f"dh{nm}")
                    nc.sync.dma_start(dh_t[:, :, :, :], brd["h" + nm][b])
                    dsw.append(dw_t)
                    dsh.append(dh_t)

                # W-convs: attn8 -> z (fp8 DoubleRow pairs, shift = 3 cols)
                for (nm, k, npair, wpad, zrows, zpad), dw_t, zt in \
                        zip(WBR, dsw, zts):
                    for c in range(2):
                        ps = PP.tile([P, 2048], f32, tag="ps", name="ps")
                        for jp in range(npair):
                            col0 = 30 + 6 * jp - wpad
                            for j in range(4):
                                r0 = 32 * c + 8 * j
                                base = attn8[:, r0:r0 + 8, col0:col0 + 64]
                                rhs = AP(base.tensor, base.offset,
                                         [base.ap[0], [3, 2], [128, 8],
                                          [1, 64]])
                                nc.tensor.matmul(
                                    ps[:, 512 * j:512 * (j + 1)],
                                    dw_t[:, jp, :, :], rhs,
                                    start=(jp == 0), stop=(jp == npair - 1),
                                    perf_mode=DR)
                        ps3 = ps.rearrange("p (a b) -> p a b", a=32)
                        r = zpad + 32 * c
                        nc.scalar.activation(
                            zt[:, r:r + 32, :], ps3[:, :, :],
                            AF.Identity, bias=0.0, scale=1.0)

                # H-convs: all branches -> one PSUM group (shift = 3 rows),
                # folded into acc with a single STT per chunk (x 1/256)
                ngrp = len(WBR)
                for c in range(2):
                    ps = PP.tile([P, 2048], f32, tag="ps", name="ps")
                    for gi, ((nm, k, npair, wpad, zrows, zpad), dh_t, zt) in \
                            enumerate(zip(WBR, dsh, zts)):
                        for jp in range(npair):
                            row0 = 6 * jp
                            for j in range(4):
                                r0 = 32 * c + 8 * j
                                base = zt[:, row0 + r0:row0 + r0 + 8, :]
                                rhs = AP(base.tensor, base.offset,
                                         [base.ap[0], [192, 2], [64, 8],
                                          [1, 64]])
                                nc.tensor.matmul(
                                    ps[:, 512 * j:512 * (j + 1)],
                                    dh_t[:, jp, :, :], rhs,
                                    start=(gi == 0 and jp == 0),
                                    stop=(gi == ngrp - 1 and
                                          jp == npair - 1),
                                    perf_mode=DR)
                    acc_c = accs[b][:, 2048 * c:2048 * (c + 1)]
                    if os.environ.get("STT", "1") == "1":
                        nc.vector.scalar_tensor_tensor(
                            acc_c, ps[:, :], 1.0 / (BR_SCALE * BR_SCALE),
                            acc_c, op0=A.mult, op1=A.add)
                    else:
                        hs = SP.tile([P, 2048], f16, tag="hs", name="hs")
                        ps3 = ps.rearrange("p (a b) -> p a b", a=32)
                        hs3 = hs.rearrange("p (a b) -> p a b", a=32)
                        for h2 in range(2):
                            nc.scalar.activation(
                                hs3[:, 16 * h2:16 * h2 + 16, :],
                                ps3[:, 16 * h2:16 * h2 + 16, :],
                                AF.Identity, bias=0.0,
                                scale=1.0 / (BR_SCALE * BR_SCALE))
                        nc.vector.tensor_tensor(acc_c, hs[:, :], acc_c,
                                                op=A.add)

            # interleave so stage-B PE work overlaps lane chains
            if os.environ.get("NO_BRANCH", "0") == "1":
                for b in range(NBLK):
                    stage_a(b)
            elif os.environ.get("INTERLEAVE", "1") == "1":
                stage_a(0)
                stage_a(1)
                stage_b(0, True)
                stage_a(2)
                stage_b(1, True)
                stage_a(3)
                stage_b(2, False)
                stage_b(3, False)
            else:
                for b in range(NBLK):
                    stage_a(b)
                for b in range(NBLK):
                    stage_b(b, b < 2)

            # ---- pointwise 1x1 conv + bias + multiply-by-x ----
            for m in range(NBLK):
                sl = slice(m * P, (m + 1) * P)
                xcan = CP.tile([P, 68, 68], f16, tag="xcan", name="xcan")
                nc.sync.dma_start(xcan[:, :, :], xc_d[sl, :, :])
                outb = SP.tile([P, HW], f16, tag="outb", name="outb")
                outb3 = outb.rearrange("p (a b) -> p a b", a=H)
                for nch in range(4):
                    ps = PP.tile([P, 2048], f32, tag="ps", name="ps")
                    for q in range(2):
                        col = (2 * nch + q) * 512
                        for kk in range(NBLK):
                            nc.tensor.matmul(
                                ps[:, 512 * q:512 * (q + 1)],
                                w3rows[kk][:, m * P:(m + 1) * P],
                                accs[kk][:, col:col + 512],
                                start=(kk == 0), stop=(kk == NBLK - 1))
                    yb = SP.tile([P, 16, 64], f16, tag="yb", name="yb")
                    nc.scalar.activation(
                        yb[:, :, :],
                        ps[:, 0:1024].rearrange("p (a b) -> p a b", a=16),
                        AF.Identity, bias=wtiles[("b3", m)][:, 0:1],
                        scale=1.0)
                    xv = xcan[:, 2 + 16 * nch:2 + 16 * nch + 16, 2:66]
                    nc.vector.tensor_tensor(
                        outb3[:, 16 * nch:16 * nch + 16, :], yb[:, :, :], xv,
                        op=A.mult)
                nc.sync.dma_start(out_d[sl, :, :], outb3[:, :, :])

    if not nc.is_finalized():
        nc.finalize()
    return nc


def _get_nc():
    global _NC
    if _NC is None:
        _NC = _build_nc()
    return _NC


def _prep_inputs(inputs):
    import ml_dtypes
    f8np = ml_dtypes.float8_e4m3

    f = lambda a, shp: np.ascontiguousarray(
        np.asarray(a, dtype=np.float32).reshape(shp))
    g = lambda nm, k: f(inputs[nm], (C, k))

    w0 = g("w0", 25)
    wd5 = np.zeros((NBLK, P, 25, P), np.float16)
    idx = np.arange(P)
    for b in range(NBLK):
        wd5[b, idx, :, idx] = w0[b * P:(b + 1) * P, :].astype(np.float16)

    def pair_stack(w, npair):
        k = w.shape[1]
        d = np.zeros((NBLK, P, npair, 2, P), np.float32)
        for b in range(NBLK):
            wb = w[b * P:(b + 1) * P, :] * BR_SCALE
            for jp in range(npair):
                for i in range(2):
                    t = 2 * jp + i
                    if t < k:
                        d[b, idx, jp, i, idx] = wb[:, t]
        return np.ascontiguousarray(d.astype(f8np))

    # branch-bias vector, folded into the pointwise bias: b3' = b3 + W3@bsumh
    bsumh = (np.asarray(inputs["b0_2"], np.float32)
             + np.asarray(inputs["b1_2"], np.float32)
             + np.asarray(inputs["b2_2"], np.float32)
             + g("w0_2", 7).sum(1) * np.asarray(inputs["b0_1"], np.float32).reshape(C)
             + g("w1_2", 11).sum(1) * np.asarray(inputs["b1_1"], np.float32).reshape(C)
             + g("w2_2", 21).sum(1) * np.asarray(inputs["b2_1"], np.float32).reshape(C))
    w3m = np.asarray(inputs["w3"], np.float32).reshape(C, C)
    b3p = np.asarray(inputs["b3"], np.float32).reshape(C) + w3m @ bsumh

    com = {
        "b0": f(inputs["b0"], (C, 1)),
        "w0sc": w0,
        "wd5": np.ascontiguousarray(wd5),
        "wdw7": pair_stack(g("w0_1", 7), 4),
        "wdh7": pair_stack(g("w0_2", 7), 4),
        "wdw11": pair_stack(g("w1_1", 11), 6),
        "wdh11": pair_stack(g("w1_2", 11), 6),
        "wdw21": pair_stack(g("w2_1", 21), 11),
        "wdh21": pair_stack(g("w2_2", 21), 11),
        "w3": np.ascontiguousarray(w3m.T.astype(np.float16)),
        "b3": np.ascontiguousarray(b3p.reshape(C, 1)),
    }
    x = np.asarray(inputs["x"], np.float32).astype(np.float16)
    xp = np.zeros((x.shape[0], C, 68, 68), np.float16)
    xp[:, :, 2:66, 2:66] = x
    return [dict(com, xcanv=np.ascontiguousarray(xp[i]))
            for i in range(x.shape[0])]


def run(inputs, trace=False):
    from concourse.bass_utils import run_bass_kernel_spmd
    nc = _get_nc()
    in_maps = _prep_inputs(inputs)
    res = run_bass_kernel_spmd(nc, in_maps, core_ids=list(range(len(in_maps))),
                               trace=trace)
    out = np.stack([r["out"] for r in res.results], axis=0).astype(np.float32)
    return out, res


def kernel(**inputs):
    out, _ = run(inputs, trace=False)
    return out

